# revision 25
# baseline (speedup 1.0000x reference)
"""Causal self-attention (B=2, T=2048, dim=2048, H=16, D=128) on 8 trn2 NeuronCores.

Sharding: data-parallel over batch (2 groups of 4 cores), tensor-parallel over
heads within a group (4 heads/core).  Each core computes its heads' QKV
projection (x @ Wqkv_part^T), RoPE, causal attention, and a partial output
projection against its W_proj column block; the host sums the 4 partials per
batch element.

v5 schedule (all matmul operands bf16, fp32 accumulation): one readiness-aware
software pipeline — attention units of window w are woven between the QKV
matmul sets of the SAME window (gated on each head's k-projection and the
v sets they consume) and the proj units of window w-1, so the ScalarE exp
stream (the attention bottleneck) always drains under QKV/proj PE work:
  - x passed transposed (dim, T); q/k produced head-transposed (d, T) so
    S^T = kT.T @ qT directly; v natural (T, d).
  - RoPE rotate-half via a PE permutation matmul (swap matrix), sign folded
    into the sin table; cos/sin kept bf16.
  - scores pipelined depth-3: PE score matmul -> ScalarE exp (bf16) -> PE
    PV; the rowsum matmul trails one unit behind (all-ones [128,128]
    stationary -> row sums replicated across partitions, no broadcast).
  - softmax normalization off the PE: reciprocal_approx_fast + multiply (DVE).
  - QKV/proj PSUM evacuation on DVE; ScalarE stays pure-Exp (one act table
    load); weights land in per-set column-slab tiles so window-0 compute
    starts as soon as its own slab arrives; y DMA'd per window.
"""

import os

import numpy as np
import ml_dtypes

import concourse.bass as bass
import concourse.bacc as bacc
import concourse.tile as tile
import concourse.mybir as mybir
from concourse import bass_utils

BF16 = mybir.dt.bfloat16
F32 = mybir.dt.float32

B, T, DIM = 2, 2048, 2048
H, D = 16, 128
HL = 4                   # heads per core
NCORES = 8
E = 3 * HL * D           # 1536 = per-core qkv output rows
NCHUNK = DIM // 128      # 16 contraction chunks
NW = T // 512            # 4 query windows
NTT = T // 128           # 16 token tiles
SCALE = 1.0 / float(np.sqrt(D))

_CACHE = {}
LAST_RESULTS = None


def _build_module():
    nc = bacc.Bacc("TRN2", target_bir_lowering=False, debug=False)
    xT = nc.dram_tensor("xT", (DIM, T), BF16, kind="ExternalInput")
    wqkvT = nc.dram_tensor("wqkvT", (DIM, E), BF16, kind="ExternalInput")
    wpT = nc.dram_tensor("wpT", (HL * D, DIM), BF16, kind="ExternalInput")
    cosT = nc.dram_tensor("cosT", (D, T), BF16, kind="ExternalInput")
    sinTs = nc.dram_tensor("sinTs", (D, T), BF16, kind="ExternalInput")
    tri = nc.dram_tensor("tri", (128, 128), BF16, kind="ExternalInput")
    swp = nc.dram_tensor("swp", (128, 128), BF16, kind="ExternalInput")
    ones = nc.dram_tensor("ones", (128, 128), BF16, kind="ExternalInput")
    y = nc.dram_tensor("y", (T, DIM), F32, kind="ExternalOutput")

    Exp = mybir.ActivationFunctionType.Exp

    with tile.TileContext(nc) as tc:
        with (
            tc.tile_pool(name="const", bufs=1) as cpool,
            tc.tile_pool(name="xp", bufs=2) as xpool,
            tc.tile_pool(name="rotp", bufs=3) as rotpool,
            tc.tile_pool(name="ptp", bufs=6) as ptpool,
            tc.tile_pool(name="rcpp", bufs=2) as rcppool,
            tc.tile_pool(name="yp", bufs=3) as ypool,
            tc.tile_pool(name="psA", bufs=2, space="PSUM") as psA,
            tc.tile_pool(name="psS", bufs=3, space="PSUM") as psS,
            tc.tile_pool(name="psO", bufs=2, space="PSUM") as psO,
            tc.tile_pool(name="psR", bufs=1, space="PSUM") as psR,
        ):
            xT_v = xT.rearrange("(c p) t -> p c t", p=128)
            wqkv_v = wqkvT.rearrange("(c p) e -> p c e", p=128)

            # per-set weight slabs: dependency granularity is per-tile, so a
            # qk set only waits for its own slab's DMA
            wq_slab = [
                cpool.tile([128, NCHUNK, 128], BF16, tag=f"wq{s}", name=f"wq{s}")
                for s in range(8)
            ]
            wv_sb = cpool.tile([128, NCHUNK, 512], BF16, tag="wv")
            wp_sb = cpool.tile([128, HL, DIM], BF16, tag="wp")
            cos_sb = cpool.tile([128, T], BF16, tag="cos")
            sin_sb = cpool.tile([128, T], BF16, tag="sin")
            tri_sb = cpool.tile([128, 128], BF16, tag="tri")
            swp_sb = cpool.tile([128, 128], BF16, tag="swp")
            ones_sb = cpool.tile([128, 128], BF16, tag="ones")
            q_sb = cpool.tile([128, HL * T], BF16, tag="q")
            k_sb = cpool.tile([128, HL * T], BF16, tag="k")
            v_sb = cpool.tile([128, NTT * HL * D], BF16, tag="v")
            o_sb = cpool.tile([128, HL * T], BF16, tag="o")

            xbs = {}

            def load_xb(w, split=False):
                xb = xpool.tile([128, NCHUNK, 512], BF16, tag="xb", name="xb")
                sl = slice(w * 512, (w + 1) * 512)
                if split:
                    for g in range(4):
                        nc.sync.dma_start(
                            xb[:, 4 * g : 4 * g + 4, :], xT_v[:, 4 * g : 4 * g + 4, sl]
                        )
                else:
                    nc.sync.dma_start(xb[:], xT_v[:, :, sl])
                xbs[w] = xb

            # Startup DMAs, first-needed first: x chunk-group 0 and weight
            # slab 0 unblock the first matmul; small tables go down the idle
            # Pool queue; the proj weights are deferred into window 0's bigs.
            xb0 = xpool.tile([128, NCHUNK, 512], BF16, tag="xb", name="xb0")
            xbs[0] = xb0
            nc.sync.dma_start(xb0[:, 0:4, :], xT_v[:, 0:4, 0:512])
            nc.sync.dma_start(wq_slab[0][:], wqkv_v[:, :, 0:128])
            for g in range(1, 4):
                nc.sync.dma_start(
                    xb0[:, 4 * g : 4 * g + 4, :], xT_v[:, 4 * g : 4 * g + 4, 0:512]
                )
            nc.sync.dma_start(wq_slab[1][:], wqkv_v[:, :, 128:256])
            nc.gpsimd.dma_start(swp_sb[:], swp[:, :])
            nc.gpsimd.dma_start(tri_sb[:], tri[:, :])
            nc.gpsimd.dma_start(ones_sb[:], ones[:, :])
            nc.gpsimd.dma_start(cos_sb[:], cosT[:, :])
            nc.gpsimd.dma_start(sin_sb[:], sinTs[:, :])
            nc.sync.dma_start(wv_sb[:], wqkv_v[:, :, 1024:1536])
            for s in range(2, 8):
                nc.sync.dma_start(wq_slab[s][:], wqkv_v[:, :, 128 * s : 128 * (s + 1)])

            def rope(src, h, w):
                sl = slice(h * T + w * 512, h * T + (w + 1) * 512)
                wsl = slice(w * 512, (w + 1) * 512)
                rp = psA.tile([128, 512], F32, tag="ps", name="rp")
                nc.tensor.matmul(rp[:], swp_sb[:], src[:, sl], start=True, stop=True)
                rot = rotpool.tile([128, 512], BF16, tag="rot", name="rot")
                nc.vector.tensor_mul(rot[:], rp[:], sin_sb[:, wsl])
                nc.vector.tensor_mul(src[:, sl], src[:, sl], cos_sb[:, wsl])
                nc.vector.tensor_add(src[:, sl], src[:, sl], rot[:])

            def qkv_bigs(w):
                """[('q'|'k'|'v', idx, closure)] — 8 qk sets then 4 v sets."""
                out = []

                def qk_set(grp, j):
                    def run():
                        xb = xbs[w]
                        dst = q_sb if grp == 0 else k_sb
                        ps = psA.tile([128, 512], F32, tag="ps", name="ps")
                        slab = wq_slab[grp * 4 + j]
                        for c in range(NCHUNK):
                            nc.tensor.matmul(
                                ps[:],
                                slab[:, c, :],
                                xb[:, c, :],
                                start=(c == 0),
                                stop=(c == NCHUNK - 1),
                            )
                        sl = slice(j * T + w * 512, j * T + (w + 1) * 512)
                        nc.vector.tensor_copy(dst[:, sl], ps[:])

                    return run

                def v_set(ttl):
                    def run():
                        xb = xbs[w]
                        ttg = w * 4 + ttl
                        ps = psA.tile([128, 512], F32, tag="ps", name="ps")
                        for c in range(NCHUNK):
                            nc.tensor.matmul(
                                ps[:],
                                xb[:, c, ttl * 128 : (ttl + 1) * 128],
                                wv_sb[:, c, :],
                                start=(c == 0),
                                stop=(c == NCHUNK - 1),
                            )
                        nc.scalar.copy(v_sb[:, ttg * 512 : (ttg + 1) * 512], ps[:])

                    return run

                # q sets, then v sets, then k sets: head h's attention stream
                # becomes ready right after its k set, one big apart, so the
                # fillers spread across the whole k/proj stretch.  Each set's
                # rope is deferred into the NEXT big so its PE swap matmul
                # never waits on the set's own PSUM evacuation.
                order = (
                    [("q", j, qk_set(0, j)) for j in range(HL)]
                    + [("v", ttl, v_set(ttl)) for ttl in range(4)]
                    + [("k", j, qk_set(1, j)) for j in range(HL)]
                )

                def chain(run, pending):
                    def go():
                        run()
                        if pending is not None:
                            pending()
                    return go

                pending = None
                for lbl, idx, run in order:
                    out.append((lbl, idx, chain(run, pending)))
                    if lbl in ("q", "k"):
                        dst = q_sb if lbl == "q" else k_sb
                        pending = (lambda d=dst, j=idx: rope(d, j, w))
                    else:
                        pending = None
                # rope of the last k set rides a dedicated tiny big
                out.append(("rk", HL - 1, pending))
                return out

            def attn_fillers(w, pos):
                """Per-(head, key-tile) closures + their readiness (number of
                this window's qkv bigs that must have been emitted first).
                The rowsum matmul for unit u is issued during unit u+1 so the
                single-bank rowsum tile's WAR on the previous head's
                reciprocal is pipeline-covered."""
                nkt = 4 * w + 4
                nu = HL * nkt
                state = {"pend": [], "issued": 0, "oT": {}, "rs": None, "rs_pend": []}

                def geom(kt):
                    if kt < 4 * w:
                        return 512 * w, 512, False
                    q0 = 128 * kt
                    return q0, 512 * (w + 1) - 128 * kt, True

                def issue_score(u):
                    h, kt = divmod(u, nkt)
                    hq = h * T
                    q0, n, diag = geom(kt)
                    st = psS.tile([128, 512], F32, tag="st", name="st")
                    nc.tensor.matmul(
                        st[:, :n],
                        k_sb[:, hq + kt * 128 : hq + (kt + 1) * 128],
                        q_sb[:, hq + q0 : hq + q0 + n],
                        start=True,
                        stop=True,
                    )
                    pt = ptpool.tile([128, 512], BF16, tag="pt", name="pt")
                    nc.scalar.activation(
                        pt[:, :n], st[:, :n], Exp, bias=0.0, scale=SCALE
                    )
                    if diag:
                        nc.vector.tensor_mul(pt[:, 0:128], pt[:, 0:128], tri_sb[:])
                    return pt, q0, n

                DEPTH = 3

                def flush_rs():
                    h, kt, pt, q0, n = state["rs_pend"].pop(0)
                    if kt == 0:
                        state["rs"] = psR.tile([128, 512], F32, tag="rs", name="rs")
                    off = q0 - 512 * w
                    nc.tensor.matmul(
                        state["rs"][:, off:512],
                        ones_sb[:],
                        pt[:, :n],
                        start=(kt == 0),
                        stop=(kt == nkt - 1),
                    )
                    if kt == nkt - 1:
                        # normalization chain for head h, all off the PE
                        hq = h * T
                        rcp = rcppool.tile([128, 512], F32, tag="rcp", name="rcp")
                        nc.vector.reciprocal_approx_fast(rcp[:], state["rs"][:])
                        nc.vector.tensor_mul(
                            o_sb[:, hq + w * 512 : hq + (w + 1) * 512],
                            state["oT"].pop(h)[:],
                            rcp[:],
                        )

                def unit(u):
                    def run():
                        h, kt = divmod(u, nkt)
                        while state["issued"] < min(u + DEPTH + 1, nu):
                            state["pend"].append(issue_score(state["issued"]))
                            state["issued"] += 1
                        if kt == 0:
                            state["oT"][h] = psO.tile(
                                [128, 512], F32, tag="oT", name="oT"
                            )
                        pt, q0, n = state["pend"].pop(0)
                        off = q0 - 512 * w
                        nc.tensor.matmul(
                            state["oT"][h][:, off:512],
                            v_sb[:, kt * 512 + h * 128 : kt * 512 + (h + 1) * 128],
                            pt[:, :n],
                            start=(kt == 0),
                            stop=(kt == nkt - 1),
                        )
                        state["rs_pend"].append((h, kt, pt, q0, n))
                        if len(state["rs_pend"]) > 1:
                            flush_rs()
                        if u == nu - 1:
                            flush_rs()

                    return run

                def score_ra(u):
                    # what the score matmul of unit u reads: this head's q,
                    # plus this window's k for diagonal tiles (+2: the rope
                    # of a set runs one big after the set itself)
                    h, kt = divmod(u, nkt)
                    if kt < 4 * w:
                        return pos[("q", h)] + 2
                    return pos[("k", h)] + 2

                def pv_ra(u):
                    h, kt = divmod(u, nkt)
                    if kt < 4 * w:
                        return 0
                    return pos[("v", kt - 4 * w)] + 1

                fillers = []
                ready = []
                for u in range(nu):
                    # emitting unit u issues scores up to u+DEPTH (lookahead),
                    # so its readiness must cover those scores' inputs too
                    ra = pv_ra(u)
                    for uu in range(u, min(u + DEPTH + 1, nu)):
                        ra = max(ra, score_ra(uu))
                    fillers.append(unit(u))
                    ready.append(ra)
                return fillers, ready

            def proj_bigs(w, split_evac=False):
                out = []

                def unit(tt, nwi, use_act):
                    def run():
                        yps = psA.tile([128, 512], F32, tag="ps", name="yps")
                        for hh in range(HL):
                            nc.tensor.matmul(
                                yps[:],
                                o_sb[:, hh * T + tt * 128 : hh * T + (tt + 1) * 128],
                                wp_sb[:, hh, nwi * 512 : (nwi + 1) * 512],
                                start=(hh == 0),
                                stop=(hh == HL - 1),
                            )
                        ysb = ypool.tile([128, 512], F32, tag="ysb", name="ysb")
                        nc.scalar.copy(ysb[:], yps[:])
                        nc.sync.dma_start(
                            y[tt * 128 : (tt + 1) * 128, nwi * 512 : (nwi + 1) * 512],
                            ysb[:],
                        )

                    return run

                i = 0
                for tt in range(4 * w, 4 * w + 4):
                    for nwi in range(DIM // 512):
                        out.append(
                            ("p", i, unit(tt, nwi, split_evac and (i % 2 == 1)))
                        )
                        i += 1
                return out

            def weave(bigs, fillers, ready):
                nb = len(bigs)
                nf = len(fillers)
                done = 0
                for i, (_, _, b) in enumerate(bigs):
                    b()
                    # cap the per-big burst so late-ready units don't flood
                    # the exp engine all at once
                    target = min(int(round(nf * (i + 1) / nb)), done + 4)
                    while done < nf and done < target and ready[done] <= i + 1:
                        fillers[done]()
                        done += 1
                while done < nf:
                    fillers[done]()
                    done += 1

            for w in range(NW):
                bigs = qkv_bigs(w)
                if w + 1 < NW:
                    # prefetch next window's x after a couple of sets
                    bigs.insert(2, ("x", w + 1, lambda wn=w + 1: load_xb(wn)))
                if w == 0:
                    bigs.append(
                        (
                            "wp",
                            0,
                            lambda: nc.sync.dma_start(
                                wp_sb[:], wpT.rearrange("(h p) n -> p h n", p=128)
                            ),
                        )
                    )
                if w >= 1:
                    bigs += proj_bigs(w - 1)
                pos = {(lbl, idx): i for i, (lbl, idx, _) in enumerate(bigs)}
                fillers, ready = attn_fillers(w, pos)
                weave(bigs, fillers, ready)
            for _, _, fn in proj_bigs(NW - 1, split_evac=True):
                fn()

    nc.compile()
    return nc


def _rope_tables():
    inv_freq = (
        1.0 / (10000.0 ** (np.arange(0, D, 2, dtype=np.float32) / np.float32(D)))
    ).astype(np.float32)
    tpos = np.arange(T, dtype=np.float32)
    freqs = tpos[:, None] * inv_freq[None, :]
    emb = np.concatenate([freqs, freqs], axis=1)  # (T, D)
    cos = np.cos(emb).astype(np.float32)
    sin = np.sin(emb).astype(np.float32)
    cosT = np.ascontiguousarray(cos.T)  # (D, T)
    sinTs = np.ascontiguousarray(sin.T)
    sinTs[0:64] *= -1.0  # fold rotate_half sign
    return (
        cosT.astype(ml_dtypes.bfloat16),
        sinTs.astype(ml_dtypes.bfloat16),
    )


def make_in_maps(x, W_qkv, W_proj):
    cosT, sinTs = _rope_tables()
    tri = (np.arange(128)[None, :] >= np.arange(128)[:, None]).astype(
        ml_dtypes.bfloat16
    )
    tri = np.ascontiguousarray(tri)
    swp = np.zeros((128, 128), dtype=ml_dtypes.bfloat16)
    idx = np.arange(64)
    swp[idx + 64, idx] = 1.0
    swp[idx, idx + 64] = 1.0
    ones = np.ones((128, 128), dtype=ml_dtypes.bfloat16)
    in_maps = []
    for c in range(NCORES):
        b, g = divmod(c, 4)
        Wq = W_qkv[512 * g : 512 * (g + 1)]
        Wk = W_qkv[2048 + 512 * g : 2048 + 512 * (g + 1)]
        Wv = W_qkv[4096 + 512 * g : 4096 + 512 * (g + 1)]
        Wc = np.concatenate([Wq, Wk, Wv], axis=0)  # (1536, 2048)
        in_maps.append(
            {
                "xT": np.ascontiguousarray(x[b].T).astype(ml_dtypes.bfloat16),
                "wqkvT": np.ascontiguousarray(Wc.T).astype(ml_dtypes.bfloat16),
                "wpT": np.ascontiguousarray(
                    W_proj[:, 512 * g : 512 * (g + 1)].T
                ).astype(ml_dtypes.bfloat16),
                "cosT": cosT,
                "sinTs": sinTs,
                "tri": tri,
                "swp": swp,
                "ones": ones,
            }
        )
    return in_maps


def kernel(x, W_qkv, W_proj):
    global LAST_RESULTS
    x = np.asarray(x, dtype=np.float32)
    W_qkv = np.asarray(W_qkv, dtype=np.float32)
    W_proj = np.asarray(W_proj, dtype=np.float32)
    assert x.shape == (B, T, DIM) and W_qkv.shape == (3 * H * D, DIM)

    if "nc" not in _CACHE:
        _CACHE["nc"] = _build_module()
    nc = _CACHE["nc"]

    in_maps = make_in_maps(x, W_qkv, W_proj)
    trace = os.environ.get("KERNEL_TRACE", "0") == "1"
    res = bass_utils.run_bass_kernel_spmd(
        nc, in_maps, core_ids=list(range(NCORES)), trace=trace
    )
    LAST_RESULTS = res
    y = np.zeros((B, T, DIM), dtype=np.float32)
    for c in range(NCORES):
        y[c // 4] += res.results[c]["y"]
    return y
